# revision 8
# baseline (speedup 1.0000x reference)
"""MultiHeadAttention Trainium2 Bass kernel (8 cores), v2.

Problem: B=2, S=2048, D=1024, H=16 heads, DK=64, fp32 in/out.
  q/k/v = x @ W* + b*; scores = q k^T / 8; attn = softmax; ctx = attn v;
  out = ctx @ Wo + bo.

Sharding (8 cores): batch (2-way) x head-group (4-way tensor parallel).
Core c handles b = c // 4 and heads [4g, 4g+4), g = c % 4 (d' slice of 256).
Host sums the 4 partials per batch and adds the bias correction
(bv @ Wo + bo).

v2 design notes (vs v1):
- All matmul operands are bf16 (cost model: bf16 moving = 1 cyc/row at ANY
  free size, vs fp32r needing >=256). psum stays f32. Total extra error
  ~0.3% rel, well under the 2e-2 gate.
- ctx is re-oriented: ctx[qi,65] += attn[kj, qi-tile]^T @ v_aug[kj, 0:65]
  with v as the 65-wide bf16 MOVING operand (65 cols incl. a ones column
  that yields the softmax denominator per qi partition). This halves ctx PE
  cost vs streaming attn (qi-wide) as the moving side, and turns the
  normalize into a cheap per-partition tensor_scalar (no partition
  broadcast).
- ctxT for the out-proj is recovered with an identity matmul on PE
  (out = ctx_norm^T @ I), 53ns per [64,128] tile.
- Unit = (cp, h, sj): qi chunk of 128, local head, kj half (8 kj-tiles).
  Per unit: 8 scores matmuls [128,128] -> exp [128,1024] on ACT -> 8 ctx
  matmuls. ACT (16.8M exps/core, ~133us) is the floor; PE (~140us) is
  dripped into the exp shadow with the proven "scores two units ahead,
  then background work, then ctx" pattern.
- PSUM: scores [128,1024]x2 (4 banks) + ctx [128,65]x2 (2) + misc
  [128,512]x2 (2) = 8 banks exactly.
"""

import numpy as np

B = 2
S = 2048
D = 1024
H = 16
DK = 64
N_CORES = 8
HL = H // 4  # 4 heads per core
DL = HL * DK  # 256 local d'
NCP = S // 128  # 16 qi chunks
NU = NCP * HL * 2  # 128 units
KT = D // 128  # 8 contraction tiles for projections

_CACHED_NC = None


def _build():
    import concourse.bacc as bacc
    import concourse.mybir as mybir
    import concourse.tile as tile

    f32 = mybir.dt.float32
    bf16 = mybir.dt.bfloat16
    Exp = mybir.ActivationFunctionType.Exp

    nc = bacc.Bacc(None)

    xT = nc.declare_dram_parameter("xT", [D, S], bf16, isOutput=False)
    wq = nc.declare_dram_parameter("wq", [D, DL], bf16, isOutput=False)
    wk = nc.declare_dram_parameter("wk", [D, DL], bf16, isOutput=False)
    wv = nc.declare_dram_parameter("wv", [D, DL], bf16, isOutput=False)
    wo = nc.declare_dram_parameter("wo", [DL, D], bf16, isOutput=False)
    bq = nc.declare_dram_parameter("bq", [128, 2], f32, isOutput=False)
    bk = nc.declare_dram_parameter("bk", [128, 2], f32, isOutput=False)
    ident = nc.declare_dram_parameter("ident", [128, 128], bf16, isOutput=False)
    out = nc.declare_dram_parameter("out", [S, D], bf16, isOutput=True)

    with tile.TileContext(nc) as tc:
        with (
            tc.tile_pool(name="persist", bufs=1) as persist,
            tc.tile_pool(name="xw", bufs=1) as xw,
            tc.tile_pool(name="attn", bufs=14) as atp,
            tc.tile_pool(name="norm", bufs=3) as npl,
            tc.tile_pool(name="rin", bufs=3) as rpl,
            tc.tile_pool(name="ob", bufs=2) as obp,
            tc.tile_pool(name="scps", bufs=2, space="PSUM") as scp,
            tc.tile_pool(name="cxps", bufs=2, space="PSUM") as cxp,
            tc.tile_pool(name="msps", bufs=2, space="PSUM") as msp,
        ):
            qT_sb = persist.tile([128, 2, S], bf16, tag="qT")
            kT_sb = persist.tile([128, 2, S], bf16, tag="kT")
            v_sb = persist.tile([128, NCP, HL, DK + 1], bf16, tag="v")
            ctxT_sb = persist.tile([128, 2, S], bf16, tag="ctxT")
            wo_sb = persist.tile([128, 2, D], bf16, tag="wo")
            bq_sb = persist.tile([128, 2], f32, tag="bq")
            bk_sb = persist.tile([128, 2], f32, tag="bk")
            id_sb = persist.tile([128, 128], bf16, tag="id")

            # Input DMAs. One DMA_ENGINES slot serializes transfers, so
            # order by need: first halves of xT (kj/qi < 1024) + wk, then wq,
            # then second halves, then wv/wo/misc.
            xt, wq_t, wk_t, wv_t = [], [], [], []
            for kt in range(KT):
                t = xw.tile([128, S], bf16, tag=f"xt{kt}")
                nc.sync.dma_start(
                    out=t[:, 0 : S // 2],
                    in_=xT[kt * 128 : (kt + 1) * 128, 0 : S // 2],
                )
                xt.append(t)
                w = xw.tile([128, DL], bf16, tag=f"wk{kt}")
                nc.sync.dma_start(out=w[:], in_=wk[kt * 128 : (kt + 1) * 128, :])
                wk_t.append(w)
            for kt in range(KT):
                w = xw.tile([128, DL], bf16, tag=f"wq{kt}")
                nc.sync.dma_start(out=w[:], in_=wq[kt * 128 : (kt + 1) * 128, :])
                wq_t.append(w)
            for kt in range(KT):
                w = xw.tile([128, DL], bf16, tag=f"wv{kt}")
                nc.sync.dma_start(out=w[:], in_=wv[kt * 128 : (kt + 1) * 128, :])
                wv_t.append(w)
            for kt in range(KT):
                nc.sync.dma_start(
                    out=xt[kt][:, S // 2 : S],
                    in_=xT[kt * 128 : (kt + 1) * 128, S // 2 : S],
                )
            for mt in range(2):
                nc.sync.dma_start(
                    out=wo_sb[:, mt, :], in_=wo[mt * 128 : (mt + 1) * 128, :]
                )
            nc.sync.dma_start(out=bq_sb[:], in_=bq[:])
            nc.sync.dma_start(out=bk_sb[:], in_=bk[:])
            nc.sync.dma_start(out=id_sb[:], in_=ident[:])
            nc.gpsimd.memset(v_sb[:, :, :, DK : DK + 1], 1.0)

            # ---- projection emitters -------------------------------------
            kT_done, qT_done, v_next = set(), set(), [0]

            def qk_chunk(which, mt, n):
                """Project one [128, 512] chunk of qT (which=0) / kT."""
                wt, dst, bias = (
                    (wq_t, qT_sb, bq_sb) if which == 0 else (wk_t, kT_sb, bk_sb)
                )
                ns = slice(n * 512, (n + 1) * 512)
                ps = msp.tile([128, 512], f32, tag="ms", name=f"pj{which}{mt}{n}")
                for kt in range(KT):
                    nc.tensor.matmul(
                        ps[:],
                        wt[kt][:, mt * 128 : (mt + 1) * 128],
                        xt[kt][:, ns],
                        start=(kt == 0),
                        stop=(kt == KT - 1),
                    )
                nc.vector.tensor_scalar_add(
                    out=dst[:, mt, ns], in0=ps[:], scalar1=bias[:, mt : mt + 1]
                )

            def ensure_kT(mt, sj):
                for n in (2 * sj, 2 * sj + 1):
                    if (mt, n) not in kT_done:
                        kT_done.add((mt, n))
                        qk_chunk(1, mt, n)

            def ensure_qT(mt, cp):
                n = cp // 4
                if (mt, n) not in qT_done:
                    qT_done.add((mt, n))
                    qk_chunk(0, mt, n)

            def v_chunk():
                jt = v_next[0]
                if jt >= NCP:
                    return
                v_next[0] = jt + 1
                ps = msp.tile([128, 512], f32, tag="ms", name=f"vc{jt}")
                for kt in range(KT):
                    nc.tensor.matmul(
                        ps[:, 0:DL],
                        xt[kt][:, jt * 128 : (jt + 1) * 128],
                        wv_t[kt][:],
                        start=(kt == 0),
                        stop=(kt == KT - 1),
                    )
                nc.vector.tensor_copy(
                    v_sb[:, jt, :, 0:DK],
                    ps[:, 0:DL].rearrange("p (h d) -> p h d", h=HL),
                )

            # ---- attention emitters --------------------------------------
            def udec(u):
                return u // 8, (u % 8) // 2, u % 2  # cp, h, sj

            sc_t, at_t, cxh, ctxn = {}, {}, {}, {}

            def emit_sc(u):
                cp, h, sj = udec(u)
                mt, hp = h // 2, h % 2
                pr = slice(64 * hp, 64 * hp + 64)
                ensure_kT(mt, sj)
                ensure_qT(mt, cp)
                sc = scp.tile([128, 1024], f32, tag="sc", name=f"sc{u}")
                for j in range(8):
                    kjt = 8 * sj + j
                    nc.tensor.matmul(
                        sc[:, j * 128 : (j + 1) * 128],
                        kT_sb[pr, mt, kjt * 128 : (kjt + 1) * 128],
                        qT_sb[pr, mt, cp * 128 : (cp + 1) * 128],
                        start=True,
                        stop=True,
                    )
                sc_t[u] = sc

            def emit_exp(u):
                at = atp.tile([128, 1024], bf16, tag="at", name=f"at{u}")
                nc.scalar.activation(at[:], sc_t.pop(u)[:], Exp, scale=0.125)
                at_t[u] = at

            def emit_ctx(u):
                cp, h, sj = udec(u)
                if sj == 0:
                    cxh[(cp, h)] = cxp.tile(
                        [128, DK + 1], f32, tag="cx", name=f"cx{cp}{h}"
                    )
                cx = cxh[(cp, h)]
                at = at_t.pop(u)
                for j in range(8):
                    kjt = 8 * sj + j
                    nc.tensor.matmul(
                        cx[:],
                        at[:, j * 128 : (j + 1) * 128],
                        v_sb[:, kjt, h, :],
                        start=(kjt == 0),
                        stop=(kjt == NCP - 1),
                    )

            def emit_norm(cp, h):
                cx = cxh.pop((cp, h))
                rinv = rpl.tile([128, 1], f32, tag="ri", name=f"ri{cp}{h}")
                nc.vector.reciprocal_approx_fast(out=rinv[:], in_=cx[:, DK : DK + 1])
                cn = npl.tile([128, DK], bf16, tag="cn", name=f"cn{cp}{h}")
                nc.vector.tensor_scalar_mul(
                    out=cn[:], in0=cx[:, 0:DK], scalar1=rinv[:, 0:1]
                )
                ctxn[(cp, h)] = cn

            tr_done = set()

            def emit_transpose(cp, mt):
                ps = msp.tile([128, 512], f32, tag="ms", name=f"tp{cp}{mt}")
                for hp in range(2):
                    cn = ctxn.pop((cp, 2 * mt + hp))
                    nc.tensor.matmul(
                        ps[64 * hp : 64 * hp + 64, 0:128],
                        cn[:],
                        id_sb[:],
                        start=True,
                        stop=True,
                    )
                nc.vector.tensor_copy(
                    ctxT_sb[:, mt, cp * 128 : (cp + 1) * 128], ps[:, 0:128]
                )
                tr_done.add((cp, mt))

            ob_t = {}

            def emit_outproj(cp, nt):
                ps = msp.tile([128, 512], f32, tag="ms", name=f"op{cp}{nt}")
                for mt in range(2):
                    nc.tensor.matmul(
                        ps[:],
                        ctxT_sb[:, mt, cp * 128 : (cp + 1) * 128],
                        wo_sb[:, mt, nt * 512 : (nt + 1) * 512],
                        start=(mt == 0),
                        stop=(mt == 1),
                    )
                if nt == 0:
                    ob = obp.tile([128, D], bf16, tag="ob", name=f"ob{cp}")
                    nc.vector.tensor_copy(ob[:, 0:512], ps[:])
                    ob_t[cp] = ob
                else:
                    ob = ob_t.pop(cp)
                    nc.vector.tensor_copy(ob[:, 512:1024], ps[:])
                    nc.sync.dma_start(
                        out=out[cp * 128 : (cp + 1) * 128, :], in_=ob[:]
                    )

            # ---- static drip schedule ------------------------------------
            # kT mt1 early; v 2 chunks/unit from u=4; qT prefetched 6/2 units
            # before each cp%4==0 boundary.
            mid = {u: [] for u in range(NU)}
            mid[0].append(lambda: ensure_qT(1, 0))
            mid[0].append(lambda: ensure_kT(1, 0))
            for u in range(5, 13):
                mid[u] += [v_chunk, v_chunk]
            for n in range(1, 4):
                mid[32 * n - 6].append(lambda n=n: ensure_qT(0, 4 * n))
                mid[32 * n - 2].append(lambda n=n: ensure_qT(1, 4 * n))

            # out-proj tasks: (cp, nt [, dma]) pending until ctxT(cp) ready
            op_pending = []

            def drain_outproj(u):
                if op_pending and (op_pending[0][0], 1) in tr_done:
                    cp, nt = op_pending.pop(0)
                    emit_outproj(cp, nt)

            # ---- main loop -----------------------------------------------
            # exp stream order: for cp0/cp1 run all sj=0 units (xT first
            # halves) before sj=1, so the first 8 exps never wait on the
            # second-half xT DMAs. ctx/normalize stay in natural unit order.
            exp_order = (
                [cp * 8 + h * 2 for cp in (0, 1, 2) for h in range(4)]
                + [cp * 8 + h * 2 + 1 for cp in (0, 1, 2) for h in range(4)]
                + list(range(24, NU))
            )
            emitted_exp = set()
            ensure_kT(0, 0)
            ensure_qT(0, 0)
            emit_sc(exp_order[0])
            emit_sc(exp_order[1])
            ctx_next = [0]

            def drain_ctx():
                while ctx_next[0] < NU:
                    uc = ctx_next[0]
                    cp, h, sj = udec(uc)
                    if uc not in emitted_exp or v_next[0] < 8 * (sj + 1):
                        return
                    emit_ctx(uc)
                    ctx_next[0] = uc + 1
                    if sj == 1:
                        emit_norm(cp, h)
                        if h % 2 == 1:
                            emit_transpose(cp, h // 2)
                            if h == 3:
                                op_pending.extend([(cp, 0), (cp, 1)])

            for i in range(NU):
                emit_exp(exp_order[i])
                emitted_exp.add(exp_order[i])
                for th in mid[i]:
                    th()
                if i + 2 < NU:
                    emit_sc(exp_order[i + 2])
                drain_ctx()
                drain_outproj(i)

            while ctx_next[0] < NU or op_pending:
                drain_ctx()
                drain_outproj(NU - 1)

    nc.compile()
    return nc


def _get_nc():
    global _CACHED_NC
    if _CACHED_NC is None:
        _CACHED_NC = _build()
    return _CACHED_NC


def _in_maps(x, Wq, bq, Wk, bk, Wv, bv, Wo, bo):
    import ml_dtypes

    bf16 = ml_dtypes.bfloat16
    xTs = [np.ascontiguousarray(x[b].T.astype(bf16)) for b in range(B)]
    ident = np.eye(128, dtype=bf16)
    maps = []
    for c in range(N_CORES):
        b, g = c // 4, c % 4
        cs = slice(g * DL, (g + 1) * DL)
        maps.append(
            {
                "xT": xTs[b],
                "wq": np.ascontiguousarray(Wq[:, cs].astype(bf16)),
                "wk": np.ascontiguousarray(Wk[:, cs].astype(bf16)),
                "wv": np.ascontiguousarray(Wv[:, cs].astype(bf16)),
                "wo": np.ascontiguousarray(Wo[cs, :].astype(bf16)),
                "bq": np.ascontiguousarray(bq[cs].reshape(2, 128).T),
                "bk": np.ascontiguousarray(bk[cs].reshape(2, 128).T),
                "ident": ident,
            }
        )
    return maps


def _assemble(results, bv, Wo, bo):
    corr = (bv.astype(np.float64) @ Wo.astype(np.float64)) + bo.astype(np.float64)
    outs = []
    for b in range(B):
        acc = np.zeros((S, D), dtype=np.float64)
        for g in range(4):
            acc += results[b * 4 + g]["out"].astype(np.float64)
        outs.append((acc + corr).astype(np.float32))
    return np.stack(outs)


def kernel(x, Wq, bq, Wk, bk, Wv, bv, Wo, bo):
    from concourse.bass_utils import run_bass_kernel_spmd

    x = np.asarray(x, dtype=np.float32)
    Wq = np.asarray(Wq, dtype=np.float32)
    Wk = np.asarray(Wk, dtype=np.float32)
    Wv = np.asarray(Wv, dtype=np.float32)
    Wo = np.asarray(Wo, dtype=np.float32)
    bq = np.asarray(bq, dtype=np.float32)
    bk = np.asarray(bk, dtype=np.float32)
    bv = np.asarray(bv, dtype=np.float32)
    bo = np.asarray(bo, dtype=np.float32)

    nc = _get_nc()
    res = run_bass_kernel_spmd(
        nc, _in_maps(x, Wq, bq, Wk, bk, Wv, bv, Wo, bo), core_ids=list(range(N_CORES))
    )
    return _assemble(res.results, bv, Wo, bo)


# revision 10
# speedup vs baseline: 1.0989x; 1.0989x over previous
"""MultiHeadAttention Trainium2 Bass kernel (8 cores), v2.

Problem: B=2, S=2048, D=1024, H=16 heads, DK=64, fp32 in/out.
  q/k/v = x @ W* + b*; scores = q k^T / 8; attn = softmax; ctx = attn v;
  out = ctx @ Wo + bo.

Sharding (8 cores): batch (2-way) x head-group (4-way tensor parallel).
Core c handles b = c // 4 and heads [4g, 4g+4), g = c % 4 (d' slice of 256).
Host sums the 4 partials per batch and adds the bias correction
(bv @ Wo + bo).

v2 design notes (vs v1):
- All matmul operands are bf16 (cost model: bf16 moving = 1 cyc/row at ANY
  free size, vs fp32r needing >=256). psum stays f32. Total extra error
  ~0.3% rel, well under the 2e-2 gate.
- ctx is re-oriented: ctx[qi,65] += attn[kj, qi-tile]^T @ v_aug[kj, 0:65]
  with v as the 65-wide bf16 MOVING operand (65 cols incl. a ones column
  that yields the softmax denominator per qi partition). This halves ctx PE
  cost vs streaming attn (qi-wide) as the moving side, and turns the
  normalize into a cheap per-partition tensor_scalar (no partition
  broadcast).
- ctxT for the out-proj is recovered with an identity matmul on PE
  (out = ctx_norm^T @ I), 53ns per [64,128] tile.
- Unit = (cp, h, sj): qi chunk of 128, local head, kj half (8 kj-tiles).
  Per unit: 8 scores matmuls [128,128] -> exp [128,1024] on ACT -> 8 ctx
  matmuls. ACT (16.8M exps/core, ~133us) is the floor; PE (~140us) is
  dripped into the exp shadow with the proven "scores two units ahead,
  then background work, then ctx" pattern.
- PSUM: scores [128,1024]x2 (4 banks) + ctx [128,65]x2 (2) + misc
  [128,512]x2 (2) = 8 banks exactly.
"""

import numpy as np

B = 2
S = 2048
D = 1024
H = 16
DK = 64
N_CORES = 8
HL = H // 4  # 4 heads per core
DL = HL * DK  # 256 local d'
NCP = S // 128  # 16 qi chunks
NU = NCP * HL * 2  # 128 units
KT = D // 128  # 8 contraction tiles for projections

_CACHED_NC = None


def _build():
    import concourse.bacc as bacc
    import concourse.mybir as mybir
    import concourse.tile as tile

    f32 = mybir.dt.float32
    bf16 = mybir.dt.bfloat16
    Exp = mybir.ActivationFunctionType.Exp

    nc = bacc.Bacc(None)

    xT = nc.declare_dram_parameter("xT", [D, S], bf16, isOutput=False)
    wq = nc.declare_dram_parameter("wq", [D, DL], bf16, isOutput=False)
    wk = nc.declare_dram_parameter("wk", [D, DL], bf16, isOutput=False)
    wv = nc.declare_dram_parameter("wv", [D, DL], bf16, isOutput=False)
    wo = nc.declare_dram_parameter("wo", [DL, D], bf16, isOutput=False)
    bq = nc.declare_dram_parameter("bq", [128, 2], f32, isOutput=False)
    bk = nc.declare_dram_parameter("bk", [128, 2], f32, isOutput=False)
    ident = nc.declare_dram_parameter("ident", [128, 128], bf16, isOutput=False)
    out = nc.declare_dram_parameter("out", [S, D], bf16, isOutput=True)

    with tile.TileContext(nc) as tc:
        with (
            tc.tile_pool(name="persist", bufs=1) as persist,
            tc.tile_pool(name="xw", bufs=1) as xw,
            tc.tile_pool(name="attn", bufs=14) as atp,
            tc.tile_pool(name="norm", bufs=3) as npl,
            tc.tile_pool(name="rin", bufs=3) as rpl,
            tc.tile_pool(name="ob", bufs=2) as obp,
            tc.tile_pool(name="scps", bufs=2, space="PSUM") as scp,
            tc.tile_pool(name="cxps", bufs=2, space="PSUM") as cxp,
            tc.tile_pool(name="msps", bufs=2, space="PSUM") as msp,
        ):
            qT_sb = persist.tile([128, 2, S], bf16, tag="qT")
            kT_sb = persist.tile([128, 2, S], bf16, tag="kT")
            v_sb = persist.tile([128, NCP, HL, DK + 1], bf16, tag="v")
            ctxT_sb = persist.tile([128, 2, S], bf16, tag="ctxT")
            wo_sb = persist.tile([128, 2, D], bf16, tag="wo")
            bq_sb = persist.tile([128, 2], f32, tag="bq")
            bk_sb = persist.tile([128, 2], f32, tag="bk")
            id_sb = persist.tile([128, 128], bf16, tag="id")

            # Input DMAs. HWDGE + DMA_ENGINES are single-slot (serialized),
            # so use few big DMAs ordered by need: wk, xT first half (kj/qi <
            # 1024), wq, wv, xT second half, wo, misc.
            xt_sb = xw.tile([128, KT, S], bf16, tag="xt")
            wk_sb = xw.tile([128, KT, DL], bf16, tag="wk")
            wq_sb = xw.tile([128, KT, DL], bf16, tag="wq")
            wv_sb = xw.tile([128, KT, DL], bf16, tag="wv")
            xt = [xt_sb[:, kt, :] for kt in range(KT)]
            wk_t = [wk_sb[:, kt, :] for kt in range(KT)]
            wq_t = [wq_sb[:, kt, :] for kt in range(KT)]
            wv_t = [wv_sb[:, kt, :] for kt in range(KT)]
            xTr = xT.rearrange("(kt p) s -> p kt s", p=128)
            nc.sync.dma_start(
                out=wk_sb[:], in_=wk.rearrange("(kt p) j -> p kt j", p=128)
            )
            nc.sync.dma_start(out=xt_sb[:, :, 0 : S // 2], in_=xTr[:, :, 0 : S // 2])
            nc.sync.dma_start(
                out=wq_sb[:], in_=wq.rearrange("(kt p) j -> p kt j", p=128)
            )
            nc.sync.dma_start(
                out=wv_sb[:], in_=wv.rearrange("(kt p) j -> p kt j", p=128)
            )
            nc.sync.dma_start(out=xt_sb[:, :, S // 2 : S], in_=xTr[:, :, S // 2 : S])
            nc.sync.dma_start(
                out=wo_sb[:], in_=wo.rearrange("(mt p) j -> p mt j", p=128)
            )
            nc.sync.dma_start(out=bq_sb[:], in_=bq[:])
            nc.sync.dma_start(out=bk_sb[:], in_=bk[:])
            nc.sync.dma_start(out=id_sb[:], in_=ident[:])
            nc.gpsimd.memset(v_sb[:, :, :, DK : DK + 1], 1.0)

            # ---- projection emitters -------------------------------------
            kT_done, qT_done, v_next = set(), set(), [0]

            def qk_chunk(which, mt, n):
                """Project one [128, 512] chunk of qT (which=0) / kT."""
                wt, dst, bias = (
                    (wq_t, qT_sb, bq_sb) if which == 0 else (wk_t, kT_sb, bk_sb)
                )
                ns = slice(n * 512, (n + 1) * 512)
                ps = msp.tile([128, 512], f32, tag="ms", name=f"pj{which}{mt}{n}")
                for kt in range(KT):
                    nc.tensor.matmul(
                        ps[:],
                        wt[kt][:, mt * 128 : (mt + 1) * 128],
                        xt[kt][:, ns],
                        start=(kt == 0),
                        stop=(kt == KT - 1),
                    )
                nc.vector.tensor_scalar_add(
                    out=dst[:, mt, ns], in0=ps[:], scalar1=bias[:, mt : mt + 1]
                )

            def ensure_kT(mt, sj):
                for n in (2 * sj, 2 * sj + 1):
                    if (mt, n) not in kT_done:
                        kT_done.add((mt, n))
                        qk_chunk(1, mt, n)

            def ensure_qT(mt, cp):
                n = cp // 4
                if (mt, n) not in qT_done:
                    qT_done.add((mt, n))
                    qk_chunk(0, mt, n)

            def v_chunk():
                jt = v_next[0]
                if jt >= NCP:
                    return
                v_next[0] = jt + 1
                ps = msp.tile([128, 512], f32, tag="ms", name=f"vc{jt}")
                for kt in range(KT):
                    nc.tensor.matmul(
                        ps[:, 0:DL],
                        xt[kt][:, jt * 128 : (jt + 1) * 128],
                        wv_t[kt],
                        start=(kt == 0),
                        stop=(kt == KT - 1),
                    )
                nc.vector.tensor_copy(
                    v_sb[:, jt, :, 0:DK],
                    ps[:, 0:DL].rearrange("p (h d) -> p h d", h=HL),
                )

            # ---- attention emitters --------------------------------------
            def udec(u):
                return u // 8, (u % 8) // 2, u % 2  # cp, h, sj

            sc_t, at_t, cxh, ctxn = {}, {}, {}, {}

            def emit_sc(u):
                cp, h, sj = udec(u)
                mt, hp = h // 2, h % 2
                pr = slice(64 * hp, 64 * hp + 64)
                ensure_kT(mt, sj)
                ensure_qT(mt, cp)
                sc = scp.tile([128, 1024], f32, tag="sc", name=f"sc{u}")
                for j in range(8):
                    kjt = 8 * sj + j
                    nc.tensor.matmul(
                        sc[:, j * 128 : (j + 1) * 128],
                        kT_sb[pr, mt, kjt * 128 : (kjt + 1) * 128],
                        qT_sb[pr, mt, cp * 128 : (cp + 1) * 128],
                        start=True,
                        stop=True,
                    )
                sc_t[u] = sc

            def emit_exp(u):
                at = atp.tile([128, 1024], bf16, tag="at", name=f"at{u}")
                nc.scalar.activation(at[:], sc_t.pop(u)[:], Exp, scale=0.125)
                at_t[u] = at

            def emit_ctx(u):
                cp, h, sj = udec(u)
                if sj == 0:
                    cxh[(cp, h)] = cxp.tile(
                        [128, DK + 1], f32, tag="cx", name=f"cx{cp}{h}"
                    )
                cx = cxh[(cp, h)]
                at = at_t.pop(u)
                for j in range(8):
                    kjt = 8 * sj + j
                    nc.tensor.matmul(
                        cx[:],
                        at[:, j * 128 : (j + 1) * 128],
                        v_sb[:, kjt, h, :],
                        start=(kjt == 0),
                        stop=(kjt == NCP - 1),
                    )

            def emit_norm(cp, h):
                cx = cxh.pop((cp, h))
                rinv = rpl.tile([128, 1], f32, tag="ri", name=f"ri{cp}{h}")
                nc.vector.reciprocal_approx_fast(out=rinv[:], in_=cx[:, DK : DK + 1])
                cn = npl.tile([128, DK], bf16, tag="cn", name=f"cn{cp}{h}")
                nc.vector.tensor_scalar_mul(
                    out=cn[:], in0=cx[:, 0:DK], scalar1=rinv[:, 0:1]
                )
                ctxn[(cp, h)] = cn

            tr_done = {}
            cur_slot = [0]

            def emit_transpose(cp, mt):
                ps = msp.tile([128, 512], f32, tag="ms", name=f"tp{cp}{mt}")
                for hp in range(2):
                    cn = ctxn.pop((cp, 2 * mt + hp))
                    nc.tensor.matmul(
                        ps[64 * hp : 64 * hp + 64, 0:128],
                        cn[:],
                        id_sb[:],
                        start=True,
                        stop=True,
                    )
                nc.vector.tensor_copy(
                    ctxT_sb[:, mt, cp * 128 : (cp + 1) * 128], ps[:, 0:128]
                )
                tr_done[(cp, mt)] = cur_slot[0]

            ob_t = {}

            def emit_outproj(cp, nt):
                ps = msp.tile([128, 512], f32, tag="ms", name=f"op{cp}{nt}")
                for mt in range(2):
                    nc.tensor.matmul(
                        ps[:],
                        ctxT_sb[:, mt, cp * 128 : (cp + 1) * 128],
                        wo_sb[:, mt, nt * 512 : (nt + 1) * 512],
                        start=(mt == 0),
                        stop=(mt == 1),
                    )
                if nt == 0:
                    ob = obp.tile([128, D], bf16, tag="ob", name=f"ob{cp}")
                    nc.vector.tensor_copy(ob[:, 0:512], ps[:])
                    ob_t[cp] = ob
                else:
                    ob = ob_t.pop(cp)
                    nc.vector.tensor_copy(ob[:, 512:1024], ps[:])
                    nc.sync.dma_start(
                        out=out[cp * 128 : (cp + 1) * 128, :], in_=ob[:]
                    )

            # ---- static drip schedule ------------------------------------
            # kT mt1 early; v 2 chunks/unit from u=4; qT prefetched 6/2 units
            # before each cp%4==0 boundary.
            mid = {u: [] for u in range(NU)}
            for u in range(5, 13):
                mid[u] += [v_chunk, v_chunk]
            for n in range(1, 4):
                mid[32 * n - 6].append(lambda n=n: ensure_qT(0, 4 * n))
                mid[32 * n - 2].append(lambda n=n: ensure_qT(1, 4 * n))

            # out-proj tasks: (cp, nt [, dma]) pending until ctxT(cp) ready
            op_pending = []

            def drain_outproj(u):
                ts_ = tr_done.get((op_pending[0][0], 1)) if op_pending else None
                if ts_ is not None and ts_ < u:
                    cp, nt = op_pending.pop(0)
                    emit_outproj(cp, nt)

            # ---- main loop -----------------------------------------------
            # exp stream order: for cp0/cp1 run all sj=0 units (xT first
            # halves) before sj=1, so the first 8 exps never wait on the
            # second-half xT DMAs. ctx/normalize stay in natural unit order.
            exp_order = (
                [cp * 8 + h * 2 for cp in (0, 1, 2) for h in range(4)]
                + [cp * 8 + h * 2 + 1 for cp in (0, 1, 2) for h in range(4)]
                + list(range(24, NU))
            )
            exp_slot = {}
            ensure_kT(0, 0)
            ensure_qT(0, 0)
            emit_sc(exp_order[0])
            emit_sc(exp_order[1])
            ctx_next = [0]

            def drain_ctx(limit):
                # Emit ctx blocks whose exp ran in a slot <= limit (one slot
                # of lag keeps the exp(u) dependency off PE's queue head).
                while ctx_next[0] < NU:
                    uc = ctx_next[0]
                    cp, h, sj = udec(uc)
                    if exp_slot.get(uc, NU) > limit or v_next[0] < 8 * (sj + 1):
                        return
                    emit_ctx(uc)
                    ctx_next[0] = uc + 1
                    if sj == 1:
                        emit_norm(cp, h)
                        if h % 2 == 1:
                            emit_transpose(cp, h // 2)
                            if h == 3:
                                op_pending.extend([(cp, 0), (cp, 1)])

            for i in range(NU):
                cur_slot[0] = i
                emit_exp(exp_order[i])
                exp_slot[exp_order[i]] = i
                drain_ctx(i - 1)
                if i + 2 < NU:
                    emit_sc(exp_order[i + 2])
                for th in mid[i]:
                    th()
                drain_outproj(i)

            cur_slot[0] = NU
            while ctx_next[0] < NU or op_pending:
                drain_ctx(NU)
                drain_outproj(NU + 1)

    nc.compile()
    return nc


def _get_nc():
    global _CACHED_NC
    if _CACHED_NC is None:
        _CACHED_NC = _build()
    return _CACHED_NC


def _in_maps(x, Wq, bq, Wk, bk, Wv, bv, Wo, bo):
    import ml_dtypes

    bf16 = ml_dtypes.bfloat16
    xTs = [np.ascontiguousarray(x[b].T.astype(bf16)) for b in range(B)]
    ident = np.eye(128, dtype=bf16)
    maps = []
    for c in range(N_CORES):
        b, g = c // 4, c % 4
        cs = slice(g * DL, (g + 1) * DL)
        maps.append(
            {
                "xT": xTs[b],
                "wq": np.ascontiguousarray(Wq[:, cs].astype(bf16)),
                "wk": np.ascontiguousarray(Wk[:, cs].astype(bf16)),
                "wv": np.ascontiguousarray(Wv[:, cs].astype(bf16)),
                "wo": np.ascontiguousarray(Wo[cs, :].astype(bf16)),
                "bq": np.ascontiguousarray(bq[cs].reshape(2, 128).T),
                "bk": np.ascontiguousarray(bk[cs].reshape(2, 128).T),
                "ident": ident,
            }
        )
    return maps


def _assemble(results, bv, Wo, bo):
    corr = (bv.astype(np.float64) @ Wo.astype(np.float64)) + bo.astype(np.float64)
    outs = []
    for b in range(B):
        acc = np.zeros((S, D), dtype=np.float64)
        for g in range(4):
            acc += results[b * 4 + g]["out"].astype(np.float64)
        outs.append((acc + corr).astype(np.float32))
    return np.stack(outs)


def kernel(x, Wq, bq, Wk, bk, Wv, bv, Wo, bo):
    from concourse.bass_utils import run_bass_kernel_spmd

    x = np.asarray(x, dtype=np.float32)
    Wq = np.asarray(Wq, dtype=np.float32)
    Wk = np.asarray(Wk, dtype=np.float32)
    Wv = np.asarray(Wv, dtype=np.float32)
    Wo = np.asarray(Wo, dtype=np.float32)
    bq = np.asarray(bq, dtype=np.float32)
    bk = np.asarray(bk, dtype=np.float32)
    bv = np.asarray(bv, dtype=np.float32)
    bo = np.asarray(bo, dtype=np.float32)

    nc = _get_nc()
    res = run_bass_kernel_spmd(
        nc, _in_maps(x, Wq, bq, Wk, bk, Wv, bv, Wo, bo), core_ids=list(range(N_CORES))
    )
    return _assemble(res.results, bv, Wo, bo)


# revision 14
# speedup vs baseline: 1.1172x; 1.0166x over previous
"""MultiHeadAttention Trainium2 Bass kernel (8 cores), v2.

Problem: B=2, S=2048, D=1024, H=16 heads, DK=64, fp32 in/out.
  q/k/v = x @ W* + b*; scores = q k^T / 8; attn = softmax; ctx = attn v;
  out = ctx @ Wo + bo.

Sharding (8 cores): batch (2-way) x head-group (4-way tensor parallel).
Core c handles b = c // 4 and heads [4g, 4g+4), g = c % 4 (d' slice of 256).
Host sums the 4 partials per batch and adds the bias correction
(bv @ Wo + bo).

v2 design notes (vs v1):
- All matmul operands are bf16 (cost model: bf16 moving = 1 cyc/row at ANY
  free size, vs fp32r needing >=256). psum stays f32. Total extra error
  ~0.3% rel, well under the 2e-2 gate.
- ctx is re-oriented: ctx[qi,65] += attn[kj, qi-tile]^T @ v_aug[kj, 0:65]
  with v as the 65-wide bf16 MOVING operand (65 cols incl. a ones column
  that yields the softmax denominator per qi partition). This halves ctx PE
  cost vs streaming attn (qi-wide) as the moving side, and turns the
  normalize into a cheap per-partition tensor_scalar (no partition
  broadcast).
- ctxT for the out-proj is recovered with an identity matmul on PE
  (out = ctx_norm^T @ I), 53ns per [64,128] tile.
- Unit = (cp, h, sj): qi chunk of 128, local head, kj half (8 kj-tiles).
  Per unit: 8 scores matmuls [128,128] -> exp [128,1024] on ACT -> 8 ctx
  matmuls. ACT (16.8M exps/core, ~133us) is the floor; PE (~140us) is
  dripped into the exp shadow with the proven "scores two units ahead,
  then background work, then ctx" pattern.
- PSUM: scores [128,1024]x2 (4 banks) + ctx [128,65]x2 (2) + misc
  [128,512]x2 (2) = 8 banks exactly.
"""

import numpy as np

B = 2
S = 2048
D = 1024
H = 16
DK = 64
N_CORES = 8
HL = H // 4  # 4 heads per core
DL = HL * DK  # 256 local d'
NCP = S // 128  # 16 qi chunks
NU = NCP * HL * 2  # 128 units
KT = D // 128  # 8 contraction tiles for projections

_CACHED_NC = None


def _build():
    import concourse.bacc as bacc
    import concourse.mybir as mybir
    import concourse.tile as tile

    f32 = mybir.dt.float32
    bf16 = mybir.dt.bfloat16
    Exp = mybir.ActivationFunctionType.Exp

    nc = bacc.Bacc(None)

    xT = nc.declare_dram_parameter("xT", [D, S], bf16, isOutput=False)
    wq = nc.declare_dram_parameter("wq", [D, DL], bf16, isOutput=False)
    wk = nc.declare_dram_parameter("wk", [D, DL], bf16, isOutput=False)
    wv = nc.declare_dram_parameter("wv", [D, DL], bf16, isOutput=False)
    wo = nc.declare_dram_parameter("wo", [DL, D], bf16, isOutput=False)
    bq = nc.declare_dram_parameter("bq", [128, 2], f32, isOutput=False)
    bk = nc.declare_dram_parameter("bk", [128, 2], f32, isOutput=False)
    ident = nc.declare_dram_parameter("ident", [128, 128], bf16, isOutput=False)
    out = nc.declare_dram_parameter("out", [S, D], bf16, isOutput=True)

    with tile.TileContext(nc) as tc:
        with (
            tc.tile_pool(name="persist", bufs=1) as persist,
            tc.tile_pool(name="xw", bufs=1) as xw,
            tc.tile_pool(name="attn", bufs=18) as atp,
            tc.tile_pool(name="norm", bufs=3) as npl,
            tc.tile_pool(name="rin", bufs=3) as rpl,
            tc.tile_pool(name="ob", bufs=2) as obp,
            tc.tile_pool(name="scps", bufs=2, space="PSUM") as scp,
            tc.tile_pool(name="cxps", bufs=2, space="PSUM") as cxp,
            tc.tile_pool(name="msps", bufs=2, space="PSUM") as msp,
        ):
            qT_sb = persist.tile([128, 2, S], bf16, tag="qT")
            kT_sb = persist.tile([128, 2, S], bf16, tag="kT")
            v_sb = persist.tile([128, NCP, HL, DK + 1], bf16, tag="v")
            ctxT_sb = persist.tile([128, 2, S], bf16, tag="ctxT")
            wo_sb = persist.tile([128, 2, D], bf16, tag="wo")
            bq_sb = persist.tile([128, 2], f32, tag="bq")
            bk_sb = persist.tile([128, 2], f32, tag="bk")
            id_sb = persist.tile([128, 128], bf16, tag="id")

            # Input DMAs. HWDGE + DMA_ENGINES are single-slot (serialized),
            # so use few big DMAs ordered by need: wk, xT first half (kj/qi <
            # 1024), wq, wv, xT second half, wo, misc.
            xt_sb = xw.tile([128, KT, S], bf16, tag="xt")
            wk_sb = xw.tile([128, KT, DL], bf16, tag="wk")
            wq_sb = xw.tile([128, KT, DL], bf16, tag="wq")
            wv_sb = xw.tile([128, KT, DL], bf16, tag="wv")
            xt = [xt_sb[:, kt, :] for kt in range(KT)]
            wk_t = [wk_sb[:, kt, :] for kt in range(KT)]
            wq_t = [wq_sb[:, kt, :] for kt in range(KT)]
            wv_t = [wv_sb[:, kt, :] for kt in range(KT)]
            xTr = xT.rearrange("(kt p) s -> p kt s", p=128)
            nc.sync.dma_start(
                out=wk_sb[:], in_=wk.rearrange("(kt p) j -> p kt j", p=128)
            )
            nc.sync.dma_start(out=xt_sb[:, :, 0 : S // 2], in_=xTr[:, :, 0 : S // 2])
            nc.sync.dma_start(
                out=wq_sb[:], in_=wq.rearrange("(kt p) j -> p kt j", p=128)
            )
            nc.sync.dma_start(
                out=wv_sb[:], in_=wv.rearrange("(kt p) j -> p kt j", p=128)
            )
            nc.sync.dma_start(out=xt_sb[:, :, S // 2 : S], in_=xTr[:, :, S // 2 : S])
            nc.sync.dma_start(
                out=wo_sb[:], in_=wo.rearrange("(mt p) j -> p mt j", p=128)
            )
            nc.sync.dma_start(out=bq_sb[:], in_=bq[:])
            nc.sync.dma_start(out=bk_sb[:], in_=bk[:])
            nc.sync.dma_start(out=id_sb[:], in_=ident[:])
            nc.gpsimd.memset(v_sb[:, :, :, DK : DK + 1], 1.0)

            # ---- projection emitters -------------------------------------
            kT_done, qT_done, v_next = set(), set(), [0, 0]

            def qk_chunk(which, mt, n):
                """Project one [128, 512] chunk of qT (which=0) / kT."""
                wt, dst, bias = (
                    (wq_t, qT_sb, bq_sb) if which == 0 else (wk_t, kT_sb, bk_sb)
                )
                ns = slice(n * 512, (n + 1) * 512)
                ps = msp.tile([128, 512], f32, tag="ms", name=f"pj{which}{mt}{n}")
                for kt in range(KT):
                    nc.tensor.matmul(
                        ps[:],
                        wt[kt][:, mt * 128 : (mt + 1) * 128],
                        xt[kt][:, ns],
                        start=(kt == 0),
                        stop=(kt == KT - 1),
                    )
                nc.vector.tensor_scalar_add(
                    out=dst[:, mt, ns], in0=ps[:], scalar1=bias[:, mt : mt + 1]
                )

            def ensure_kT(mt, sj):
                for n in (2 * sj, 2 * sj + 1):
                    if (mt, n) not in kT_done:
                        kT_done.add((mt, n))
                        qk_chunk(1, mt, n)

            def ensure_qT(mt, cp):
                n = cp // 4
                if (mt, n) not in qT_done:
                    qT_done.add((mt, n))
                    qk_chunk(0, mt, n)

            def v_chunk(mt):
                jt = v_next[mt]
                if jt >= NCP:
                    return
                v_next[mt] = jt + 1
                ps = msp.tile([128, 512], f32, tag="ms", name=f"vc{mt}{jt}")
                for kt in range(KT):
                    nc.tensor.matmul(
                        ps[:, 0:128],
                        xt[kt][:, jt * 128 : (jt + 1) * 128],
                        wv_t[kt][:, mt * 128 : (mt + 1) * 128],
                        start=(kt == 0),
                        stop=(kt == KT - 1),
                    )
                nc.vector.tensor_copy(
                    v_sb[:, jt, 2 * mt : 2 * mt + 2, 0:DK],
                    ps[:, 0:128].rearrange("p (h d) -> p h d", h=2),
                )

            # ---- attention emitters --------------------------------------
            def udec(u):
                return u // 8, (u % 8) // 2, u % 2  # cp, h, sj

            sc_t, at_t, cxh, ctxn = {}, {}, {}, {}

            def emit_sc(u):
                cp, h, sj = udec(u)
                mt, hp = h // 2, h % 2
                pr = slice(64 * hp, 64 * hp + 64)
                ensure_kT(mt, sj)
                ensure_qT(mt, cp)
                sc = scp.tile([128, 1024], f32, tag="sc", name=f"sc{u}")
                for j in range(8):
                    kjt = 8 * sj + j
                    nc.tensor.matmul(
                        sc[:, j * 128 : (j + 1) * 128],
                        kT_sb[pr, mt, kjt * 128 : (kjt + 1) * 128],
                        qT_sb[pr, mt, cp * 128 : (cp + 1) * 128],
                        start=True,
                        stop=True,
                    )
                sc_t[u] = sc

            def emit_exp(u):
                at = atp.tile([128, 1024], bf16, tag="at", name=f"at{u}")
                nc.scalar.activation(at[:], sc_t.pop(u)[:], Exp, scale=0.125)
                at_t[u] = at

            def emit_ctx(u):
                cp, h, sj = udec(u)
                if sj == 0:
                    cxh[(cp, h)] = cxp.tile(
                        [128, DK + 1], f32, tag="cx", name=f"cx{cp}{h}"
                    )
                cx = cxh[(cp, h)]
                at = at_t.pop(u)
                for j in range(8):
                    kjt = 8 * sj + j
                    nc.tensor.matmul(
                        cx[:],
                        at[:, j * 128 : (j + 1) * 128],
                        v_sb[:, kjt, h, :],
                        start=(kjt == 0),
                        stop=(kjt == NCP - 1),
                    )

            def emit_norm(cp, h):
                cx = cxh.pop((cp, h))
                rinv = rpl.tile([128, 1], f32, tag="ri", name=f"ri{cp}{h}")
                nc.vector.reciprocal_approx_fast(out=rinv[:], in_=cx[:, DK : DK + 1])
                cn = npl.tile([128, DK], bf16, tag="cn", name=f"cn{cp}{h}")
                nc.vector.tensor_scalar_mul(
                    out=cn[:], in0=cx[:, 0:DK], scalar1=rinv[:, 0:1]
                )
                ctxn[(cp, h)] = cn

            tr_done = {}
            cur_slot = [0]

            def emit_transpose(cp, mt):
                ps = msp.tile([128, 512], f32, tag="ms", name=f"tp{cp}{mt}")
                for hp in range(2):
                    cn = ctxn.pop((cp, 2 * mt + hp))
                    nc.tensor.matmul(
                        ps[64 * hp : 64 * hp + 64, 0:128],
                        cn[:],
                        id_sb[:],
                        start=True,
                        stop=True,
                    )
                nc.vector.tensor_copy(
                    ctxT_sb[:, mt, cp * 128 : (cp + 1) * 128], ps[:, 0:128]
                )
                tr_done[(cp, mt)] = cur_slot[0]

            ob_t = {}

            def emit_outproj(cp, nt):
                ps = msp.tile([128, 512], f32, tag="ms", name=f"op{cp}{nt}")
                for mt in range(2):
                    nc.tensor.matmul(
                        ps[:],
                        ctxT_sb[:, mt, cp * 128 : (cp + 1) * 128],
                        wo_sb[:, mt, nt * 512 : (nt + 1) * 512],
                        start=(mt == 0),
                        stop=(mt == 1),
                    )
                if nt == 0:
                    ob = obp.tile([128, D], bf16, tag="ob", name=f"ob{cp}")
                    nc.vector.tensor_copy(ob[:, 0:512], ps[:])
                    ob_t[cp] = ob
                else:
                    ob = ob_t.pop(cp)
                    nc.vector.tensor_copy(ob[:, 512:1024], ps[:])
                    nc.sync.dma_start(
                        out=out[cp * 128 : (cp + 1) * 128, :], in_=ob[:]
                    )

            # ---- static drip schedule ------------------------------------
            # mt-phase exp order: all mt0 units (h0,h1 over every cp) first,
            # then all mt1 units. mt1's projections (kT(1), v(1), qT(1,0))
            # drip across the whole mt0 phase; out-proj (full 2-mt
            # contraction, ctxT(mt0) persists in SBUF) drips into the mt1
            # phase. Inside phase mt0 the first 4 cps run sj=0 before sj=1 so
            # the first 16 exps never wait on the second-half xT DMA.
            def mku(cp, h, sj):
                return cp * 8 + h * 2 + sj

            exp_order = (
                [mku(cp, h, 0) for cp in range(4) for h in (0, 1)]
                + [mku(cp, h, 1) for cp in range(4) for h in (0, 1)]
                + [
                    mku(cp, h, sj)
                    for cp in range(4, NCP)
                    for h in (0, 1)
                    for sj in (0, 1)
                ]
                + [
                    mku(cp, h, sj)
                    for cp in range(NCP)
                    for h in (2, 3)
                    for sj in (0, 1)
                ]
            )
            ctx_order = [
                mku(cp, h, sj)
                for hs in ((0, 1), (2, 3))
                for cp in range(NCP)
                for h in hs
                for sj in (0, 1)
            ]
            mid = {u: [] for u in range(NU)}
            for u in range(3, 11):  # v(mt0): 2/slot
                mid[u] += [lambda: v_chunk(0), lambda: v_chunk(0)]
            for u in range(36, 52):  # v(mt1): 1/slot
                mid[u].append(lambda: v_chunk(1))
            mid[12].append(lambda: ensure_qT(0, 4))
            mid[28].append(lambda: ensure_qT(0, 8))
            mid[44].append(lambda: ensure_qT(0, 12))
            mid[52].append(lambda: ensure_kT(1, 0))
            mid[58].append(lambda: ensure_kT(1, 1))
            mid[62].append(lambda: ensure_qT(1, 0))
            mid[76].append(lambda: ensure_qT(1, 4))
            mid[92].append(lambda: ensure_qT(1, 8))
            mid[108].append(lambda: ensure_qT(1, 12))

            # out-proj tasks: (cp, nt) pending until ctxT(cp) drained
            op_pending = []

            def drain_outproj(u):
                ts_ = tr_done.get((op_pending[0][0], 1)) if op_pending else None
                if ts_ is not None and ts_ < u:
                    cp, nt = op_pending.pop(0)
                    emit_outproj(cp, nt)

            # ---- main loop -----------------------------------------------
            exp_slot = {}
            ensure_kT(0, 0)
            ensure_qT(0, 0)
            emit_sc(exp_order[0])
            emit_sc(exp_order[1])
            ctx_next = [0]

            def drain_ctx(limit):
                # Emit ctx blocks whose exp ran in a slot <= limit (one slot
                # of lag keeps the exp(u) dependency off PE's queue head).
                while ctx_next[0] < NU:
                    uc = ctx_order[ctx_next[0]]
                    cp, h, sj = udec(uc)
                    if (
                        exp_slot.get(uc, NU) > limit
                        or v_next[h // 2] < 8 * (sj + 1)
                    ):
                        return
                    emit_ctx(uc)
                    ctx_next[0] += 1
                    if sj == 1:
                        emit_norm(cp, h)
                        if h % 2 == 1:
                            emit_transpose(cp, h // 2)
                            if h == 3:
                                op_pending.extend([(cp, 0), (cp, 1)])

            for i in range(NU):
                cur_slot[0] = i
                emit_exp(exp_order[i])
                exp_slot[exp_order[i]] = i
                drain_ctx(i - 1)
                if i + 2 < NU:
                    emit_sc(exp_order[i + 2])
                for th in mid[i]:
                    th()
                drain_outproj(i)

            cur_slot[0] = NU
            while ctx_next[0] < NU or op_pending:
                drain_ctx(NU)
                drain_outproj(NU + 1)

    nc.compile()
    return nc


def _get_nc():
    global _CACHED_NC
    if _CACHED_NC is None:
        _CACHED_NC = _build()
    return _CACHED_NC


def _in_maps(x, Wq, bq, Wk, bk, Wv, bv, Wo, bo):
    import ml_dtypes

    bf16 = ml_dtypes.bfloat16
    xTs = [np.ascontiguousarray(x[b].T.astype(bf16)) for b in range(B)]
    ident = np.eye(128, dtype=bf16)
    maps = []
    for c in range(N_CORES):
        b, g = c // 4, c % 4
        cs = slice(g * DL, (g + 1) * DL)
        maps.append(
            {
                "xT": xTs[b],
                "wq": np.ascontiguousarray(Wq[:, cs].astype(bf16)),
                "wk": np.ascontiguousarray(Wk[:, cs].astype(bf16)),
                "wv": np.ascontiguousarray(Wv[:, cs].astype(bf16)),
                "wo": np.ascontiguousarray(Wo[cs, :].astype(bf16)),
                "bq": np.ascontiguousarray(bq[cs].reshape(2, 128).T),
                "bk": np.ascontiguousarray(bk[cs].reshape(2, 128).T),
                "ident": ident,
            }
        )
    return maps


def _assemble(results, bv, Wo, bo):
    corr = (bv.astype(np.float64) @ Wo.astype(np.float64)) + bo.astype(np.float64)
    outs = []
    for b in range(B):
        acc = np.zeros((S, D), dtype=np.float64)
        for g in range(4):
            acc += results[b * 4 + g]["out"].astype(np.float64)
        outs.append((acc + corr).astype(np.float32))
    return np.stack(outs)


def kernel(x, Wq, bq, Wk, bk, Wv, bv, Wo, bo):
    from concourse.bass_utils import run_bass_kernel_spmd

    x = np.asarray(x, dtype=np.float32)
    Wq = np.asarray(Wq, dtype=np.float32)
    Wk = np.asarray(Wk, dtype=np.float32)
    Wv = np.asarray(Wv, dtype=np.float32)
    Wo = np.asarray(Wo, dtype=np.float32)
    bq = np.asarray(bq, dtype=np.float32)
    bk = np.asarray(bk, dtype=np.float32)
    bv = np.asarray(bv, dtype=np.float32)
    bo = np.asarray(bo, dtype=np.float32)

    nc = _get_nc()
    res = run_bass_kernel_spmd(
        nc, _in_maps(x, Wq, bq, Wk, bk, Wv, bv, Wo, bo), core_ids=list(range(N_CORES))
    )
    return _assemble(res.results, bv, Wo, bo)
